# revision 64
# baseline (speedup 1.0000x reference)
"""Trainium2 Bass kernel for nn_Attention (B=4, T=1024, C=1024, 16 heads).

Sharding: 8 cores = (batch b, query-half q). Core i handles queries
t in [q*512, q*512+512) of batch b = i//2, computing K/V for the whole
batch locally (33% redundant FLOPs but zero collectives — far cheaper
than any on-device all-reduce at this size). Host gather is a pure
concatenation/transpose.

Everything on-chip is laid out so no transposes are ever needed:
  - the host passes x^T (tokens rotated so the query half comes first;
    key order is a permutation, which softmax attention is invariant to)
  - Q^T, K^T come out of their projections in [o, t] layout directly
  - V is produced in [t, o] layout with a leading ones-column per head,
    so the PV matmul's PSUM row 0 is the softmax denominator Z for free
  - softmax runs on S^T = (QK^T)^T (keys on partitions, queries on the
    free dim) with no max-subtraction (logits are O(6), exp is safe)
  - normalization: fast custom-DVE reciprocal of the Z row (partition
    0), GpSimd partition_broadcast, one DVE multiply — no TensorE work
  - the output projection consumes out^T [c2, t] directly and yields
    y^T + bias (per-partition bias on ScalarE); host transposes back.

Schedule (single pass, no loops): K/Q/S for head-pair 0 first (its
weight column loads in halves ahead of x^T, so TensorE starts ~4 us in), then the V
stage (streaming behind the x^T/Wv DMAs, Wv o-halves split so the
first V pass starts earlier), then per head-pair group: K^T chunk,
Q^T chunk, 16 paired S matmuls into 2-bank PSUM tiles (one [128,1024]
exp covers both heads, halving ScalarE overhead), PV + the
normalization chain. All eight Wproj columns prefetch one-per-group
through the Wv pool slots (dead after the V stage), so the output
projection is pure compute; a few keep-warm matmuls bridge the final
norm chains so it runs at 2.4 GHz.

All matmuls use float32r (TF32-like fast path: 1 cycle/row at N=512,
same speed as bf16 but ~1.5e-4 element error instead of ~4e-3) with
fp32 PSUM accumulation. End-to-end rel err vs the fp32 reference is
~4e-4; measured HW exec time ~195-196 us (from 399 us for the first
working version). Steady-state matmul issue rate sits at the 227
ns/matmul hardware floor; the residue over the ~160 us ideal is the
shared-HBM-limited load stream (~10 us) and the fixed kernel-tail
drain/barrier (~10 us).
"""

import numpy as np

B, T, C = 4, 1024, 1024
NH, HD = 16, 64
TQ = T // 2
KC = C // 128  # 8 contraction chunks
SCALE = 1.0 / float(np.sqrt(HD))

_PROG = None
import os
USE_BF16 = os.environ.get("KERNEL_BF16", "0") == "1"


def _build_program():
    import concourse.bacc as bacc
    import concourse.mybir as mybir
    import concourse.tile as tile

    F32 = mybir.dt.float32
    F32R = mybir.dt.bfloat16 if USE_BF16 else mybir.dt.float32r
    Exp = mybir.ActivationFunctionType.Exp
    Ident = mybir.ActivationFunctionType.Identity

    def r(ap):
        return ap.bitcast(F32R)

    nc = bacc.Bacc()
    xt_d = nc.declare_dram_parameter("xt", [KC, 128, T], F32R, isOutput=False)
    wq_d = nc.declare_dram_parameter("wq", [KC, 128, C], F32R, isOutput=False)
    wk_d = nc.declare_dram_parameter("wk", [KC, 128, C], F32R, isOutput=False)
    wv_d = nc.declare_dram_parameter("wv", [KC, 128, C], F32R, isOutput=False)
    wp_d = nc.declare_dram_parameter("wp", [KC, 128, C], F32R, isOutput=False)
    bias_d = nc.declare_dram_parameter("bias", [128, 8], F32, isOutput=False)
    ones_d = nc.declare_dram_parameter("ones", [128, 128], F32R, isOutput=False)
    yt_d = nc.declare_dram_parameter("yt", [8, 128, TQ], F32, isOutput=True)

    from contextlib import ExitStack

    with ExitStack() as ctx:
        tc = ctx.enter_context(tile.TileContext(nc))
        ctx.enter_context(
            nc.allow_low_precision(
                "float32r matmul inputs (TF32-like) are intentional"
            )
        )
        pool = lambda name, bufs, **kw: ctx.enter_context(  # noqa: E731
            tc.tile_pool(name=name, bufs=bufs, **kw)
        )
        xt_pool = pool("xt", KC)
        wstat_pool = pool("wstat", 3)
        wmov_pool = pool("wmov", KC)
        kt_pool = pool("kt", 2)
        qt_pool = pool("qt", 2)
        v_pool = pool("vaug", KC)
        exp_pool = pool("exp", 9)
        ot_pool = pool("ot", KC)
        y_pool = pool("ysb", 2)
        osb2_pool = pool("osb2", 2)
        r_pool = pool("rsb", 1)
        rbx_pool = pool("rbx", 2)
        bias_pool = pool("bias", 8)
        ps_proj = pool("psproj", 2, space="PSUM")
        ps_s = pool("pss", 2, space="PSUM")  # [128,1024] pair tiles, 2 banks each
        ps_ops = pool("psops", 2, space="PSUM")
        if True:
            # ---- stage 0: streamed loads ------------------------------
            # xt first (every projection contracts over all of it), then
            # the first group's weights, then wv for the V stage.
            kw0 = wstat_pool.tile([128, C], F32R, tag="wstat", name="kw0")
            nc.sync.dma_start(kw0[:, 0:TQ], wk_d[0][:, 0:TQ])
            nc.sync.dma_start(kw0[:, TQ:C], wk_d[0][:, TQ:C])
            xt = []
            for k in range(KC):
                t_ = xt_pool.tile([128, T], F32R, tag="xt", name=f"xt{k}")
                nc.sync.dma_start(t_[:], xt_d[k])
                xt.append(t_)
                if k == 0:
                    qw0 = wstat_pool.tile([128, C], F32R, tag="wstat", name="qw0")
                    nc.sync.dma_start(qw0[:], wq_d[0])

            # wv streamed in o-halves: n=0 halves interleave with xt so
            # the first V pass starts early; n=1 halves follow.
            wv_sb = []
            for k in range(KC):
                wvt = wmov_pool.tile([128, C], F32R, tag="wmov", name=f"wv{k}")
                nc.sync.dma_start(wvt[:, 0:TQ], wv_d[k][:, 0:TQ])
                wv_sb.append(wvt)
            for k in range(KC):
                nc.sync.dma_start(wv_sb[k][:, TQ:C], wv_d[k][:, TQ:C])

            ones_sb = bias_pool.tile([128, NH], F32R, tag="ones", name="ones_sb")
            nc.sync.dma_start(ones_sb[:], ones_d[:, 0:NH])
            va = []
            for m in range(KC):
                vt = v_pool.tile([128, NH * 65], F32R, tag="vaug", name=f"va{m}")
                view = vt[:].rearrange("p (h e) -> p h e", e=65)
                # ones column FIRST per head: the PV output's Z row lands
                # on partition 0 (the only base the custom-DVE reciprocal
                # and GpSimd partition_broadcast support).
                nc.vector.tensor_copy(view[:, :, 0:1], ones_sb[:].unsqueeze(2))
                va.append(vt)

            bias_t = bias_pool.tile([128, 8], F32, tag="bias", name="bias_t")
            nc.sync.dma_start(bias_t[:], bias_d[:])
            bias_sb = [bias_t[:, m:m + 1] for m in range(8)]

            ot = []
            for k in range(KC):
                o_ = ot_pool.tile([128, TQ], F32R, tag="ot", name=f"ot{k}")
                ot.append(o_)
            ob2_last = []

            def emit_kt(kc, kw):
                k_ = kt_pool.tile([128, T], F32R, tag="kt", name=f"kt{kc}")
                for n in range(2):
                    ps = ps_proj.tile([128, 512], F32, tag="ps", name=f"k{kc}{n}")
                    for k in range(KC):
                        nc.tensor.matmul(
                            ps[:], r(kw[:, k * 128:(k + 1) * 128]),
                            r(xt[k][:, n * 512:(n + 1) * 512]),
                            start=(k == 0), stop=(k == KC - 1),
                        )
                    nc.vector.tensor_copy(k_[:, n * 512:(n + 1) * 512], ps[:])
                return k_

            def emit_qt(kc, qw):
                ps = ps_proj.tile([128, TQ], F32, tag="ps", name=f"q{kc}")
                for k in range(KC):
                    nc.tensor.matmul(
                        ps[:], r(qw[:, k * 128:(k + 1) * 128]), r(xt[k][:, 0:TQ]),
                        start=(k == 0), stop=(k == KC - 1),
                    )
                q_ = qt_pool.tile([128, TQ], F32R, tag="qt", name=f"qt{kc}")
                nc.vector.tensor_copy(q_[:], ps[:])
                return q_

            def emit_s_pair(kc, k_, q_):
                # Both heads' S^T chunk j share one 2-bank PSUM tile so a
                # single [128,1024] exp covers them (halves ACT overhead).
                exps = []
                for j in range(KC):
                    sps = ps_s.tile([128, 2 * TQ], F32, tag="ps", name=f"s{kc}{j}")
                    nc.tensor.matmul(
                        sps[:, 0:TQ],
                        r(k_[0:64, j * 128:(j + 1) * 128]),
                        r(q_[0:64, :]),
                        start=True, stop=True,
                    )
                    nc.tensor.matmul(
                        sps[:, TQ:2 * TQ],
                        r(k_[64:128, j * 128:(j + 1) * 128]),
                        r(q_[64:128, :]),
                        start=True, stop=True,
                    )
                    e = exp_pool.tile([128, 2 * TQ], F32R, tag="exp",
                                      name=f"e{kc}{j}")
                    nc.scalar.activation(e[:], sps[:], Exp, scale=SCALE)
                    exps.append(e)
                return exps

            def emit_norm(h, ops):
                okc, half = divmod(h, 2)
                po = half * 64
                # Fast 1/Z straight from the PSUM Z row (partition 0),
                # broadcast across partitions on idle GpSimd, one ScalarE
                # copy and one DVE multiply. No PE work at all.
                rt0 = r_pool.tile([1, TQ], F32, tag="rsb", name=f"r0{h}")
                nc.vector.reciprocal_approx_fast(rt0[0:1, :], ops[0:1, :])
                rbx = rbx_pool.tile([65, TQ], F32, tag="rbx", name=f"rbx{h}")
                nc.gpsimd.partition_broadcast(rbx[:], rt0[0:1, :])
                ob2 = osb2_pool.tile([65, TQ], F32R, tag="osb2", name=f"ob2_{h}")
                nc.vector.tensor_mul(ob2[:], ops[0:65, :], rbx[:])
                nc.sync.dma_start(ot[okc][po:po + 64, :], ob2[1:65, :])
                ob2_last.append(ob2)

            def emit_pv(h, exps, lo, pre_norm=None):
                ops = ps_ops.tile([65, TQ], F32, tag="ps", name=f"o{h}")
                for j in range(KC):
                    nc.tensor.matmul(
                        ops[:], r(va[j][:, h * 65:(h + 1) * 65]),
                        r(exps[j][:, lo:lo + TQ]),
                        start=(j == 0), stop=(j == KC - 1),
                    )
                if pre_norm is not None:
                    pre_norm()
                emit_norm(h, ops)

            # ---- group 0 header: K/Q/S for heads 0,1 (needs only xt) --
            kt0 = emit_kt(0, kw0)
            qt0 = emit_qt(0, qw0)
            exps0 = emit_s_pair(0, kt0, qt0)

            # ---- V = x @ Wv^T ([t,o] + ones cols), streams behind DMA --
            for n in range(2):
                for m in range(KC):
                    view = va[m][:].rearrange("p (h e) -> p h e", e=65)
                    ps = ps_proj.tile([128, 512], F32, tag="ps", name=f"v{m}{n}")
                    for k in range(KC):
                        nc.tensor.matmul(
                            ps[:], r(xt[k][:, m * 128:(m + 1) * 128]),
                            r(wv_sb[k][:, n * 512:(n + 1) * 512]),
                            start=(k == 0), stop=(k == KC - 1),
                        )
                    src = ps[:].rearrange("p (h d) -> p h d", d=64)
                    nc.vector.tensor_copy(view[:, n * 8:(n + 1) * 8, 1:65], src)

            # ---- remaining SDPA groups ------------------------------
            emit_pv(0, exps0, 0)
            emit_pv(1, exps0, TQ)
            yw_sb = {}
            y_head = {}
            for kc in range(1, KC):
                kw = wstat_pool.tile([128, C], F32R, tag="wstat", name=f"kw{kc}")
                nc.sync.dma_start(kw[:], wk_d[kc])
                k_ = emit_kt(kc, kw)
                qw = wstat_pool.tile([128, C], F32R, tag="wstat", name=f"qw{kc}")
                nc.sync.dma_start(qw[:], wq_d[kc])
                q_ = emit_qt(kc, qw)
                exps = emit_s_pair(kc, k_, q_)
                emit_pv(2 * kc, exps, 0)
                emit_pv(2 * kc + 1, exps, TQ)
                # prefetch Wproj column(s) through the wv slots (free
                # after the V stage) so the Y stage is pure compute
                m = kc - 1
                yw = wmov_pool.tile([128, C], F32R, tag="wmov", name=f"yw{m}")
                nc.sync.dma_start(yw[:], wp_d[m])
                yw_sb[m] = yw

            # Keep TensorE's HAM activity window busy while the final
            # norm chains drain, so the Y stage starts at 2.4 GHz.
            warm = ps_proj.tile([128, TQ], F32, tag="ps", name="warm")
            for w in range(4):
                nc.tensor.matmul(
                    warm[:], r(xt[w % KC][:, 0:128]), r(xt[(w + 1) % KC][:, 0:TQ]),
                    start=(w == 0), stop=(w == 3),
                )
            warm2 = ps_ops.tile([128, TQ], F32, tag="ps", name="warm2")
            for w in range(4):
                nc.tensor.matmul(
                    warm2[:], r(xt[w % KC][:, 0:128]), r(xt[(w + 1) % KC][:, 0:TQ]),
                    start=(w == 0), stop=(w == 3),
                )

            for m in (7,):  # remaining Wproj column
                yw = wmov_pool.tile([128, C], F32R, tag="wmov", name=f"yw{m}")
                nc.sync.dma_start(yw[:], wp_d[m])
                yw_sb[m] = yw

            # ---- y^T = Wproj @ out^T + b ------------------------------
            for m in range(8):
                yw = yw_sb[m]
                ps = ps_proj.tile([128, TQ], F32, tag="ps", name=f"y{m}")
                for k in range(KC):
                    nc.tensor.matmul(
                        ps[:], r(yw[:, k * 128:(k + 1) * 128]), r(ot[k][:]),
                        start=(k == 0), stop=(k == KC - 1),
                    )
                ysb = y_pool.tile([128, TQ], F32, tag="ysb", name=f"ysb{m}")
                nc.scalar.activation(ysb[:], ps[:], Ident, bias=bias_sb[m])
                nc.sync.dma_start(yt_d[m], ysb[:])

    nc.compile()
    return nc


def _get_program():
    global _PROG
    if _PROG is None:
        _PROG = _build_program()
    return _PROG


def _prep_inputs(x, Wqkv, Wproj, bproj):
    """Host-side shard prep: per-core input maps (all fp32 contiguous)."""
    x = np.asarray(x, dtype=np.float32)
    Wqkv = np.asarray(Wqkv, dtype=np.float32)
    Wproj = np.asarray(Wproj, dtype=np.float32)
    bproj = np.asarray(bproj, dtype=np.float32)

    def cols(wT):
        # [C, C] (c, o) -> [8, 128, 1024]: per o-chunk column, laid out
        # so one contiguous 512KB DMA fills the SBUF stationary tile
        # [128p, k*128+d] = wT[k*128+p, ko*128+d]
        return np.ascontiguousarray(
            wT.reshape(KC, 128, 8, 128).transpose(2, 1, 0, 3).reshape(8, 128, C)
        )

    mmdt = np.float32
    if USE_BF16:
        import ml_dtypes

        mmdt = ml_dtypes.bfloat16
    wq = cols(Wqkv[0:C].T).astype(mmdt)
    wk = cols(Wqkv[C:2 * C].T).astype(mmdt)
    wv = np.ascontiguousarray(Wqkv[2 * C:3 * C].T.reshape(KC, 128, C)).astype(mmdt)
    wp = cols(Wproj.T).astype(mmdt)
    bias = np.ascontiguousarray(bproj.reshape(8, 128).T)
    ones = np.ones((128, 128), dtype=mmdt)

    in_maps = []
    for i in range(8):
        b, q = divmod(i, 2)
        xb = x[b]
        if q == 0:
            rot = xb
        else:
            rot = np.concatenate([xb[TQ:], xb[:TQ]], axis=0)
        xt = np.ascontiguousarray(rot.T.reshape(KC, 128, T)).astype(mmdt)
        in_maps.append(
            {
                "xt": xt, "wq": wq, "wk": wk, "wv": wv, "wp": wp,
                "bias": bias, "ones": ones,
            }
        )
    return in_maps


def _assemble(results, x_dtype):
    out = np.empty((B, T, C), dtype=np.float32)
    for i in range(8):
        b, q = divmod(i, 2)
        yt = results[i]["yt"]  # [8, 128, TQ] = y^T chunked over o
        out[b, q * TQ:(q + 1) * TQ, :] = yt.reshape(C, TQ).T
    return out.astype(x_dtype, copy=False)


def run(inputs, trace=False, **spmd_kwargs):
    """Shared entry for kernel() and test harnesses (trace for profiling)."""
    from concourse.bass_utils import run_bass_kernel_spmd

    nc = _get_program()
    in_maps = _prep_inputs(**inputs)
    res = run_bass_kernel_spmd(
        nc, in_maps, list(range(8)), trace=trace, **spmd_kwargs
    )
    out = _assemble(res.results, np.asarray(inputs["x"]).dtype)
    return out, res


def kernel(x, Wqkv, Wproj, bproj):
    out, _ = run(dict(x=x, Wqkv=Wqkv, Wproj=Wproj, bproj=bproj))
    return out


# revision 65
# speedup vs baseline: 1.0142x; 1.0142x over previous
"""Trainium2 Bass kernel for nn_Attention (B=4, T=1024, C=1024, 16 heads).

Sharding: 8 cores = (batch b, query-half q). Core i handles queries
t in [q*512, q*512+512) of batch b = i//2, computing K/V for the whole
batch locally (33% redundant FLOPs but zero collectives — far cheaper
than any on-device all-reduce at this size). Host gather is a pure
concatenation/transpose.

Everything on-chip is laid out so no transposes are ever needed:
  - the host passes x^T (tokens rotated so the query half comes first;
    key order is a permutation, which softmax attention is invariant to)
  - Q^T, K^T come out of their projections in [o, t] layout directly
  - V is produced in [t, o] layout with a leading ones-column per head,
    so the PV matmul's PSUM row 0 is the softmax denominator Z for free
  - softmax runs on S^T = (QK^T)^T (keys on partitions, queries on the
    free dim) with no max-subtraction (logits are O(6), exp is safe)
  - normalization: fast custom-DVE reciprocal of the Z row (partition
    0), GpSimd partition_broadcast, one DVE multiply — no TensorE work
  - the output projection consumes out^T [c2, t] directly and yields
    y^T + bias (per-partition bias on ScalarE); host transposes back.

Schedule (single pass, no loops): K/Q/S for head-pair 0 first (its
weight column loads in halves ahead of x^T, so TensorE starts ~4 us in), then the V
stage (streaming behind the x^T/Wv DMAs, Wv o-halves split so the
first V pass starts earlier), then per head-pair group: K^T chunk,
Q^T chunk, 16 paired S matmuls into 2-bank PSUM tiles (one [128,1024]
exp covers both heads, halving ScalarE overhead), PV + the
normalization chain. All eight Wproj columns prefetch one-per-group
through the Wv pool slots (dead after the V stage), so the output
projection is pure compute; a few keep-warm matmuls bridge the final
norm chains so it runs at 2.4 GHz.

All matmuls use float32r (TF32-like fast path: 1 cycle/row at N=512,
same speed as bf16 but ~1.5e-4 element error instead of ~4e-3) with
fp32 PSUM accumulation. End-to-end rel err vs the fp32 reference is
~4e-4; measured HW exec time ~195-196 us (from 399 us for the first
working version). Steady-state matmul issue rate sits at the 227
ns/matmul hardware floor; the residue over the ~160 us ideal is the
shared-HBM-limited load stream (~10 us) and the fixed kernel-tail
drain/barrier (~10 us).
"""

import numpy as np

B, T, C = 4, 1024, 1024
NH, HD = 16, 64
TQ = T // 2
KC = C // 128  # 8 contraction chunks
SCALE = 1.0 / float(np.sqrt(HD))

_PROG = None
import os
USE_BF16 = os.environ.get("KERNEL_BF16", "0") == "1"


def _build_program():
    import concourse.bacc as bacc
    import concourse.mybir as mybir
    import concourse.tile as tile

    F32 = mybir.dt.float32
    F32R = mybir.dt.bfloat16 if USE_BF16 else mybir.dt.float32r
    Exp = mybir.ActivationFunctionType.Exp
    Ident = mybir.ActivationFunctionType.Identity

    def r(ap):
        return ap.bitcast(F32R)

    nc = bacc.Bacc()
    xt_d = nc.declare_dram_parameter("xt", [KC, 128, T], F32R, isOutput=False)
    wq_d = nc.declare_dram_parameter("wq", [KC, 128, C], F32R, isOutput=False)
    wk_d = nc.declare_dram_parameter("wk", [KC, 128, C], F32R, isOutput=False)
    wv_d = nc.declare_dram_parameter("wv", [KC, 128, C], F32R, isOutput=False)
    wp_d = nc.declare_dram_parameter("wp", [KC, 128, C], F32R, isOutput=False)
    bias_d = nc.declare_dram_parameter("bias", [128, 8], F32, isOutput=False)
    ones_d = nc.declare_dram_parameter("ones", [128, 128], F32R, isOutput=False)
    yt_d = nc.declare_dram_parameter("yt", [8, 128, TQ], F32, isOutput=True)

    from contextlib import ExitStack

    with ExitStack() as ctx:
        tc = ctx.enter_context(tile.TileContext(nc))
        ctx.enter_context(
            nc.allow_low_precision(
                "float32r matmul inputs (TF32-like) are intentional"
            )
        )
        pool = lambda name, bufs, **kw: ctx.enter_context(  # noqa: E731
            tc.tile_pool(name=name, bufs=bufs, **kw)
        )
        xt_pool = pool("xt", KC)
        wstat_pool = pool("wstat", 3)
        wmov_pool = pool("wmov", KC)
        kt_pool = pool("kt", 2)
        qt_pool = pool("qt", 2)
        v_pool = pool("vaug", KC)
        exp_pool = pool("exp", 9)
        ot_pool = pool("ot", KC)
        y_pool = pool("ysb", 2)
        osb2_pool = pool("osb2", 2)
        r_pool = pool("rsb", 1)
        rbx_pool = pool("rbx", 2)
        bias_pool = pool("bias", 8)
        ps_proj = pool("psproj", 2, space="PSUM")
        ps_s = pool("pss", 2, space="PSUM")  # [128,1024] pair tiles, 2 banks each
        ps_ops = pool("psops", 2, space="PSUM")
        if True:
            # ---- stage 0: streamed loads ------------------------------
            # xt first (every projection contracts over all of it), then
            # the first group's weights, then wv for the V stage.
            kw0 = wstat_pool.tile([128, C], F32R, tag="wstat", name="kw0")
            nc.sync.dma_start(kw0[:, 0:TQ], wk_d[0][:, 0:TQ])
            nc.sync.dma_start(kw0[:, TQ:C], wk_d[0][:, TQ:C])
            xt = []
            for k in range(KC):
                t_ = xt_pool.tile([128, T], F32R, tag="xt", name=f"xt{k}")
                nc.sync.dma_start(t_[:], xt_d[k])
                xt.append(t_)
                if k == 0:
                    qw0 = wstat_pool.tile([128, C], F32R, tag="wstat", name="qw0")
                    nc.sync.dma_start(qw0[:], wq_d[0])

            # wv streamed in o-halves: n=0 halves interleave with xt so
            # the first V pass starts early; n=1 halves follow.
            wv_sb = []
            for k in range(KC):
                wvt = wmov_pool.tile([128, C], F32R, tag="wmov", name=f"wv{k}")
                nc.sync.dma_start(wvt[:, 0:TQ], wv_d[k][:, 0:TQ])
                wv_sb.append(wvt)
            for k in range(KC):
                nc.sync.dma_start(wv_sb[k][:, TQ:C], wv_d[k][:, TQ:C])

            ones_sb = bias_pool.tile([128, NH], F32R, tag="ones", name="ones_sb")
            nc.sync.dma_start(ones_sb[:], ones_d[:, 0:NH])
            va = []
            for m in range(KC):
                vt = v_pool.tile([128, NH * 65], F32R, tag="vaug", name=f"va{m}")
                view = vt[:].rearrange("p (h e) -> p h e", e=65)
                # ones column FIRST per head: the PV output's Z row lands
                # on partition 0 (the only base the custom-DVE reciprocal
                # and GpSimd partition_broadcast support).
                nc.vector.tensor_copy(view[:, :, 0:1], ones_sb[:].unsqueeze(2))
                va.append(vt)

            bias_t = bias_pool.tile([128, 8], F32, tag="bias", name="bias_t")
            nc.sync.dma_start(bias_t[:], bias_d[:])
            bias_sb = [bias_t[:, m:m + 1] for m in range(8)]

            ot = []
            for k in range(KC):
                o_ = ot_pool.tile([128, TQ], F32R, tag="ot", name=f"ot{k}")
                ot.append(o_)
            ob2_last = []

            def emit_kt(kc, kw):
                k_ = kt_pool.tile([128, T], F32R, tag="kt", name=f"kt{kc}")
                for n in range(2):
                    ps = ps_proj.tile([128, 512], F32, tag="ps", name=f"k{kc}{n}")
                    for k in range(KC):
                        nc.tensor.matmul(
                            ps[:], r(kw[:, k * 128:(k + 1) * 128]),
                            r(xt[k][:, n * 512:(n + 1) * 512]),
                            start=(k == 0), stop=(k == KC - 1),
                        )
                    nc.vector.tensor_copy(k_[:, n * 512:(n + 1) * 512], ps[:])
                return k_

            def emit_qt(kc, qw):
                ps = ps_proj.tile([128, TQ], F32, tag="ps", name=f"q{kc}")
                for k in range(KC):
                    nc.tensor.matmul(
                        ps[:], r(qw[:, k * 128:(k + 1) * 128]), r(xt[k][:, 0:TQ]),
                        start=(k == 0), stop=(k == KC - 1),
                    )
                q_ = qt_pool.tile([128, TQ], F32R, tag="qt", name=f"qt{kc}")
                nc.vector.tensor_copy(q_[:], ps[:])
                return q_

            def emit_s_pair(kc, k_, q_):
                # Both heads' S^T chunk j share one 2-bank PSUM tile so a
                # single [128,1024] exp covers them (halves ACT overhead).
                exps = []
                for j in range(KC):
                    sps = ps_s.tile([128, 2 * TQ], F32, tag="ps", name=f"s{kc}{j}")
                    nc.tensor.matmul(
                        sps[:, 0:TQ],
                        r(k_[0:64, j * 128:(j + 1) * 128]),
                        r(q_[0:64, :]),
                        start=True, stop=True,
                    )
                    nc.tensor.matmul(
                        sps[:, TQ:2 * TQ],
                        r(k_[64:128, j * 128:(j + 1) * 128]),
                        r(q_[64:128, :]),
                        start=True, stop=True,
                    )
                    e = exp_pool.tile([128, 2 * TQ], F32R, tag="exp",
                                      name=f"e{kc}{j}")
                    nc.scalar.activation(e[:], sps[:], Exp, scale=SCALE)
                    exps.append(e)
                return exps

            def emit_norm(h, ops):
                okc, half = divmod(h, 2)
                po = half * 64
                # Fast 1/Z straight from the PSUM Z row (partition 0),
                # broadcast across partitions on idle GpSimd, one ScalarE
                # copy and one DVE multiply. No PE work at all.
                rt0 = r_pool.tile([1, TQ], F32, tag="rsb", name=f"r0{h}")
                nc.vector.reciprocal_approx_fast(rt0[0:1, :], ops[0:1, :])
                rbx = rbx_pool.tile([65, TQ], F32, tag="rbx", name=f"rbx{h}")
                nc.gpsimd.partition_broadcast(rbx[:], rt0[0:1, :])
                ob2 = osb2_pool.tile([65, TQ], F32R, tag="osb2", name=f"ob2_{h}")
                nc.vector.tensor_mul(ob2[:], ops[0:65, :], rbx[:])
                nc.sync.dma_start(ot[okc][po:po + 64, :], ob2[1:65, :])
                ob2_last.append(ob2)

            def emit_pv(h, exps, lo, pre_norm=None):
                ops = ps_ops.tile([65, TQ], F32, tag="ps", name=f"o{h}")
                for j in range(KC):
                    nc.tensor.matmul(
                        ops[:], r(va[j][:, h * 65:(h + 1) * 65]),
                        r(exps[j][:, lo:lo + TQ]),
                        start=(j == 0), stop=(j == KC - 1),
                    )
                if pre_norm is not None:
                    pre_norm()
                emit_norm(h, ops)

            # ---- group 0 header: K/Q/S for heads 0,1 (needs only xt) --
            kt0 = emit_kt(0, kw0)
            qt0 = emit_qt(0, qw0)
            exps0 = emit_s_pair(0, kt0, qt0)

            # ---- V = x @ Wv^T ([t,o] + ones cols), streams behind DMA --
            for n in range(2):
                for m in range(KC):
                    view = va[m][:].rearrange("p (h e) -> p h e", e=65)
                    ps = ps_proj.tile([128, 512], F32, tag="ps", name=f"v{m}{n}")
                    for k in range(KC):
                        nc.tensor.matmul(
                            ps[:], r(xt[k][:, m * 128:(m + 1) * 128]),
                            r(wv_sb[k][:, n * 512:(n + 1) * 512]),
                            start=(k == 0), stop=(k == KC - 1),
                        )
                    src = ps[:].rearrange("p (h d) -> p h d", d=64)
                    nc.vector.tensor_copy(view[:, n * 8:(n + 1) * 8, 1:65], src)

            # ---- remaining SDPA groups ------------------------------
            emit_pv(0, exps0, 0)
            emit_pv(1, exps0, TQ)
            yw_sb = {}
            y_head = {}
            for kc in range(1, KC):
                kw = wstat_pool.tile([128, C], F32R, tag="wstat", name=f"kw{kc}")
                nc.sync.dma_start(kw[:], wk_d[kc])
                k_ = emit_kt(kc, kw)
                qw = wstat_pool.tile([128, C], F32R, tag="wstat", name=f"qw{kc}")
                nc.sync.dma_start(qw[:], wq_d[kc])
                q_ = emit_qt(kc, qw)
                exps = emit_s_pair(kc, k_, q_)
                emit_pv(2 * kc, exps, 0)
                emit_pv(2 * kc + 1, exps, TQ)
                # prefetch Wproj column(s) through the wv slots (free
                # after the V stage) so the Y stage is pure compute
                m = kc - 1
                yw = wmov_pool.tile([128, C], F32R, tag="wmov", name=f"yw{m}")
                nc.sync.dma_start(yw[:], wp_d[m])
                yw_sb[m] = yw

            # Keep TensorE's HAM activity window busy while the final
            # norm chains drain, so the Y stage starts at 2.4 GHz.
            warm = ps_ops.tile([128, TQ], F32, tag="ps", name="warm")
            for w in range(8):
                nc.tensor.matmul(
                    warm[:], r(xt[w % KC][:, 0:128]), r(xt[(w + 1) % KC][:, 0:TQ]),
                    start=(w == 0), stop=(w == 7),
                )

            for m in (7,):  # remaining Wproj column
                yw = wmov_pool.tile([128, C], F32R, tag="wmov", name=f"yw{m}")
                nc.sync.dma_start(yw[:], wp_d[m])
                yw_sb[m] = yw

            # ---- y^T = Wproj @ out^T + b ------------------------------
            for m in range(8):
                yw = yw_sb[m]
                ps = ps_proj.tile([128, TQ], F32, tag="ps", name=f"y{m}")
                for k in range(KC):
                    nc.tensor.matmul(
                        ps[:], r(yw[:, k * 128:(k + 1) * 128]), r(ot[k][:]),
                        start=(k == 0), stop=(k == KC - 1),
                    )
                ysb = y_pool.tile([128, TQ], F32, tag="ysb", name=f"ysb{m}")
                nc.scalar.activation(ysb[:], ps[:], Ident, bias=bias_sb[m])
                nc.sync.dma_start(yt_d[m], ysb[:])

    nc.compile()
    return nc


def _get_program():
    global _PROG
    if _PROG is None:
        _PROG = _build_program()
    return _PROG


def _prep_inputs(x, Wqkv, Wproj, bproj):
    """Host-side shard prep: per-core input maps (all fp32 contiguous)."""
    x = np.asarray(x, dtype=np.float32)
    Wqkv = np.asarray(Wqkv, dtype=np.float32)
    Wproj = np.asarray(Wproj, dtype=np.float32)
    bproj = np.asarray(bproj, dtype=np.float32)

    def cols(wT):
        # [C, C] (c, o) -> [8, 128, 1024]: per o-chunk column, laid out
        # so one contiguous 512KB DMA fills the SBUF stationary tile
        # [128p, k*128+d] = wT[k*128+p, ko*128+d]
        return np.ascontiguousarray(
            wT.reshape(KC, 128, 8, 128).transpose(2, 1, 0, 3).reshape(8, 128, C)
        )

    mmdt = np.float32
    if USE_BF16:
        import ml_dtypes

        mmdt = ml_dtypes.bfloat16
    wq = cols(Wqkv[0:C].T).astype(mmdt)
    wk = cols(Wqkv[C:2 * C].T).astype(mmdt)
    wv = np.ascontiguousarray(Wqkv[2 * C:3 * C].T.reshape(KC, 128, C)).astype(mmdt)
    wp = cols(Wproj.T).astype(mmdt)
    bias = np.ascontiguousarray(bproj.reshape(8, 128).T)
    ones = np.ones((128, 128), dtype=mmdt)

    in_maps = []
    for i in range(8):
        b, q = divmod(i, 2)
        xb = x[b]
        if q == 0:
            rot = xb
        else:
            rot = np.concatenate([xb[TQ:], xb[:TQ]], axis=0)
        xt = np.ascontiguousarray(rot.T.reshape(KC, 128, T)).astype(mmdt)
        in_maps.append(
            {
                "xt": xt, "wq": wq, "wk": wk, "wv": wv, "wp": wp,
                "bias": bias, "ones": ones,
            }
        )
    return in_maps


def _assemble(results, x_dtype):
    out = np.empty((B, T, C), dtype=np.float32)
    for i in range(8):
        b, q = divmod(i, 2)
        yt = results[i]["yt"]  # [8, 128, TQ] = y^T chunked over o
        out[b, q * TQ:(q + 1) * TQ, :] = yt.reshape(C, TQ).T
    return out.astype(x_dtype, copy=False)


def run(inputs, trace=False, **spmd_kwargs):
    """Shared entry for kernel() and test harnesses (trace for profiling)."""
    from concourse.bass_utils import run_bass_kernel_spmd

    nc = _get_program()
    in_maps = _prep_inputs(**inputs)
    res = run_bass_kernel_spmd(
        nc, in_maps, list(range(8)), trace=trace, **spmd_kwargs
    )
    out = _assemble(res.results, np.asarray(inputs["x"]).dtype)
    return out, res


def kernel(x, Wqkv, Wproj, bproj):
    out, _ = run(dict(x=x, Wqkv=Wqkv, Wproj=Wproj, bproj=bproj))
    return out
